# Initial kernel scaffold
#
"""Multi-head attention forward (B=4, N=2048, C=1024, H=16) on 8 Trainium2 cores.

Sharding: (batch, head-half) across 8 cores. Core c handles batch b = c//2 and
heads g*8..g*8+8 where g = c%2. Each core computes qkv for its head slice,
attention for its 8 heads, and a partial output projection over its 512
input-channel slice. The host sums the two partial projections per batch
(the tensor-parallel all-reduce) and adds b_proj.

On-chip dataflow (per core):
  - Phase A: q, k, v for ALL query blocks. q/k are produced transposed
    ([dims, tokens]); q is stored PACKED per head-pair ([128, N]: head even
    on partitions 0-63, head odd on 64-127) and staged per pair into
    pre-zeroed [128, NQB] tiles so the score matmuls run at full K=128
    (K=64 sub-array matmuls stream ~45% slower on hw).
  - v is produced in natural [key, d] layout with a fused ones column per
    head, so the P@V matmul also produces softmax denominators (PSUM row 64).
  - Phase B is software-pipelined one group deep: the Tensor queue per
    128x2-key group g is [S(g), inject/norm slot, PV(g-1)], so the exp of
    group g (ScalarE, 1/sqrt(hd) scale folded in, no max-subtraction —
    scores ~ N(0,8^2) cannot overflow exp in f32) overlaps S(g+1)/PV(g-1)
    and the Tensor engine never waits on an exp in queue order.
  - Normalization is DEFERRED off the Tensor critical path: PSUM copy +
    reciprocal_approx_fast (DVE, partition-0 input only) at the pair
    boundary, then K=1 ones-matmul broadcasts + DVE multiplies two groups
    into the NEXT pair's stream.
  - The projection matmuls of block nb-1 are interleaved two-at-a-time into
    the kc2 in {3..6} slots of block nb's groups (after the kc2==2 norm slot
    so each chunk's p=3 step follows outHT[3]'s write, and chunk PSUM accs
    are never mid-flight across the norm slot's bc allocations).
"""

import sys

if "/opt/trn_rl_repo" not in sys.path:
    sys.path.insert(0, "/opt/trn_rl_repo")

import numpy as np

B, N, C = 4, 2048, 1024
H, HD = 16, 64
NCORES = 8
HLOC = H // 2          # heads per core
PAIRS = HLOC // 2      # head-pair tiles per core
CIN = HLOC * HD        # 512: proj input slice per core
NQB = 512              # query-block width
NBLK = N // NQB        # 4
CCH = C // 128         # 8 contraction chunks for the projections
KCH = N // 128         # 16 key chunks

MM_DT_NAME = "float32r"  # "float32r" (tf32-class) or "bfloat16" (~30us faster, fails accuracy)

_BUILD_CACHE = {}


def _build(mm_dt_name, debug=False):
    import concourse.mybir as mybir
    import concourse.tile as tile
    from concourse import bacc

    DT = getattr(mybir.dt, mm_dt_name)
    F32 = mybir.dt.float32
    AF = mybir.ActivationFunctionType

    nc = bacc.Bacc(None, target_bir_lowering=False)
    xT = nc.dram_tensor("xT", [C, N], DT, kind="ExternalInput")
    wqkT = nc.dram_tensor("wqkT", [C, 2 * CIN], DT, kind="ExternalInput")
    wvT = nc.dram_tensor("wvT", [C, CIN], DT, kind="ExternalInput")
    wpT = nc.dram_tensor("wpT", [CIN, C], DT, kind="ExternalInput")
    yT = nc.dram_tensor("yT", [C, N], F32, kind="ExternalOutput")
    dbg = None
    if debug:
        dbg = {
            "qp0": nc.dram_tensor("d_qp0", [128, N], F32, kind="ExternalOutput"),
            "kT0": nc.dram_tensor("d_kT0", [128, N], F32, kind="ExternalOutput"),
            "v0": nc.dram_tensor("d_v0", [128, (HLOC + 1) * (HD + 1)], F32, kind="ExternalOutput"),
            "et000": nc.dram_tensor("d_et000", [128, 2 * NQB], F32, kind="ExternalOutput"),
            "pv00": nc.dram_tensor("d_pv00", [HD + 1, NQB], F32, kind="ExternalOutput"),
            "pv01": nc.dram_tensor("d_pv01", [HD + 1, NQB], F32, kind="ExternalOutput"),
            "rec00": nc.dram_tensor("d_rec00", [1, NQB], F32, kind="ExternalOutput"),
            "out00": nc.dram_tensor("d_out00", [128, NQB], F32, kind="ExternalOutput"),
        }

    with nc.allow_low_precision(reason="softmax intermediates kept in matmul dtype"):
        with tile.TileContext(nc) as tc:
            _emit(nc, tc, tile, mybir, DT, F32, AF, xT, wqkT, wvT, wpT, yT, dbg)
    nc.compile()
    return nc


def _emit(nc, tc, tile, mybir, DT, F32, AF, xT, wqkT, wvT, wpT, yT, dbg=None):
    from contextlib import ExitStack

    ctx = ExitStack()
    with ctx:
        persist = ctx.enter_context(tc.tile_pool(name="persist", bufs=1))
        # "big" slots ([128,1024]): wqk weights in phase A, then rotate to
        # et (exp) tiles in phase B.
        big = ctx.enter_context(tc.tile_pool(name="big", bufs=8))
        # "mid" slots ([*,512] = 2KB/partition): x chunks in phase A, then the
        # normalize-chain temporaries and yt staging in phase B.
        mid = ctx.enter_context(tc.tile_pool(name="mid", bufs=17))
        # "o" slots ([128,512]): wv weight chunks in phase A, outHT in phase B
        outp = ctx.enter_context(tc.tile_pool(name="outs", bufs=8))
        ps_s = ctx.enter_context(tc.tile_pool(name="ps_s", bufs=2, space="PSUM"))
        ps_v = ctx.enter_context(tc.tile_pool(name="ps_v", bufs=2, space="PSUM"))
        ps_acc = ctx.enter_context(tc.tile_pool(name="ps_acc", bufs=2, space="PSUM"))

        # --- persistent tiles ---------------------------------------------
        # q packed per pair: head 2p on partitions 0-63, head 2p+1 on 64-127.
        qp = [persist.tile([128, N], DT, tag=f"qp{p}", name=f"qp{p}") for p in range(PAIRS)]
        kT = [persist.tile([128, N], DT, tag=f"kT{p}", name=f"kT{p}") for p in range(PAIRS)]
        # v with a fused ones column per head: [key_chunk][128, HLOC, HD+1]
        # free size padded to even so the uint32-bitcast tail memset divides
        v_sb = [persist.tile([128, (HLOC + 1) * (HD + 1) + 1], DT, tag=f"v{kc}", name=f"v{kc}") for kc in range(KCH)]
        # K=1 broadcast stationary for the denominator-broadcast matmuls
        ones_m = persist.tile([1, HD], DT, tag="ones_m")
        ones_f32 = persist.tile([128, HLOC], F32, tag="ones_f32")
        wp_sb = [persist.tile([128, C], DT, tag=f"wp{p}", name=f"wp{p}") for p in range(PAIRS)]
        # zero-padded q staging for the score matmuls: K=64 sub-array matmuls
        # stream ~45% slower than K=128 on hw, so q is staged per pair into
        # these pre-zeroed tiles (head e on partitions 64e..64e+64, rest 0)
        # and S runs as a full-K matmul against the 2-head kT pair stationary.
        # Two ping-pong sets (even/odd pair), staged one pair ahead.
        qz_stage = [persist.tile([128, NQB], DT, tag=f"qs{i}", name=f"qs{i}") for i in range(4)]

        nc.vector.memset(ones_f32[:], 1.0)
        nc.vector.tensor_copy(ones_m[:], ones_f32[0:1, 0:1].broadcast_to((1, HD)))
        for i in range(4):
            nc.vector.memset(qz_stage[i][:].bitcast(mybir.dt.uint32), 0)
        for kc in range(KCH):
            v3 = v_sb[kc][:, 0:HLOC * (HD + 1)].rearrange("p (h d) -> p h d", h=HLOC)
            nc.vector.tensor_copy(v3[:, :, HD], ones_f32[:, 0:HLOC])
            # zero tail pad so head 7's 128-wide stationary window reads zeros
            nc.vector.memset(v_sb[kc][:, HLOC * (HD + 1):].bitcast(mybir.dt.uint32), 0)

        # --- phase A DMA: wv + x(0) first (v matmuls start earliest), then
        # wqk; wp (persist) is fetched after the phase A stream is queued.
        wv_sb = [outp.tile([128, CIN], DT, tag="o", name=f"wv{ci}") for ci in range(CCH)]
        xt_blk = {0: [mid.tile([128, NQB], DT, tag="mid", name="xt0") for _ in range(CCH)]}
        for ci in range(CCH):
            nc.sync.dma_start(wv_sb[ci][:], wvT[ci * 128:(ci + 1) * 128, :])
            nc.sync.dma_start(xt_blk[0][ci][:], xT[ci * 128:(ci + 1) * 128, 0:NQB])
        wqk_sb = [big.tile([128, 2 * CIN], DT, tag="big", name=f"wqk{ci}") for ci in range(CCH)]
        for ci in range(CCH):
            nc.sync.dma_start(wqk_sb[ci][:], wqkT[ci * 128:(ci + 1) * 128, :])

        def wv_mv(ci):
            return wv_sb[ci][:]

        # --- phase A: all of q, k, v --------------------------------------
        for nb in range(NBLK):
            nsl = slice(nb * NQB, (nb + 1) * NQB)
            xt = xt_blk.pop(nb)
            if nb + 1 < NBLK:
                nxt = []
                for ci in range(CCH):
                    t = mid.tile([128, NQB], DT, tag="mid", name="xt")
                    nc.sync.dma_start(t[:], xT[ci * 128:(ci + 1) * 128,
                                               (nb + 1) * NQB:(nb + 2) * NQB])
                    nxt.append(t)
                xt_blk[nb + 1] = nxt
            # v first (block 0's weights arrive first)
            for j in range(NQB // 128):
                kc = nb * (NQB // 128) + j
                acc = ps_acc.tile([128, CIN], F32, tag="acc")
                for ci in range(CCH):
                    nc.tensor.matmul(
                        acc[:], xt[ci][:, j * 128:(j + 1) * 128], wv_mv(ci),
                        start=(ci == 0), stop=(ci == CCH - 1),
                    )
                v3 = v_sb[kc][:, 0:HLOC * (HD + 1)].rearrange("p (h d) -> p h d", h=HLOC)
                nc.vector.tensor_copy(
                    v3[:, :, 0:HD],
                    acc[:].rearrange("p (h d) -> p h d", h=HLOC),
                )
            # k (dt 4-7 -> kT) before q (dt 0-3 -> qp): phase B's first S
            # matmul waits on kT's last write, so k must not trail phase A
            for dt_i in list(range(4, 8)) + list(range(4)):
                acc = ps_acc.tile([128, NQB], F32, tag="acc")
                for ci in range(CCH):
                    nc.tensor.matmul(
                        acc[:], wqk_sb[ci][:, dt_i * 128:(dt_i + 1) * 128], xt[ci][:],
                        start=(ci == 0), stop=(ci == CCH - 1),
                    )
                if dt_i < PAIRS:
                    nc.vector.tensor_copy(qp[dt_i][:, nsl], acc[:])
                else:
                    nc.vector.tensor_copy(kT[dt_i - PAIRS][:, nsl], acc[:])

        # wp fetch: lands during block 0's attention, needed first at block 1
        # (proj of block 0).
        for pch in range(CIN // 128):
            nc.sync.dma_start(wp_sb[pch][:], wpT[pch * 128:(pch + 1) * 128, :])

        if dbg is not None:
            nc.sync.dma_start(dbg["qp0"][:, :], qp[0][:].bitcast(F32))
            nc.sync.dma_start(dbg["kT0"][:, :], kT[0][:].bitcast(F32))
            nc.sync.dma_start(dbg["v0"][:, :], v_sb[0][:].bitcast(F32))

        # --- phase B: attention + deferred normalize + interleaved proj ---
        def make_proj_items(outHT_prev, nsl_prev, pools=None):
            items = []
            for ct in range(C // 128):
                def gen(ct=ct):
                    pool, tag = (pools[ct] if pools else (ps_acc, "acc"))
                    acc = pool.tile([128, NQB], F32, tag=tag, name="pacc")
                    for p in range(PAIRS):
                        nc.tensor.matmul(
                            acc[:], wp_sb[p][:, ct * 128:(ct + 1) * 128],
                            outHT_prev[p][:],
                            start=(p == 0), stop=(p == PAIRS - 1),
                        )
                        if p < PAIRS - 1:
                            yield
                    yt = mid.tile([128, NQB], F32, tag="mid", name="yt")
                    nc.vector.tensor_copy(yt[:], acc[:])
                    nc.sync.dma_start(yT[ct * 128:(ct + 1) * 128, nsl_prev], yt[:])
                    yield
                items.append(gen(ct))
            return items

        def finish_norm(pn, e):
            # one head per call so the two bc matmuls land in separate groups
            # (both stay under the exp pace). matmul dst partition base must
            # be 0, so each head gets its own [64, NQB] PSUM tile.
            nb_of, p, outHT_t, pv_sbs, recs = pn
            if DT is F32:
                rec_dt = recs[e]
            else:
                rec_dt = mid.tile([1, NQB], DT, tag="mid", name="rec_dt")
                nc.vector.tensor_copy(rec_dt[:], recs[e][:])
            bc = ps_acc.tile([HD, NQB], F32, tag="acc", name="bc")
            nc.tensor.matmul(bc[:], ones_m[:], rec_dt[:], start=True, stop=True)
            nc.vector.tensor_mul(
                outHT_t[p][64 * e:64 * e + HD, :], pv_sbs[e][0:HD, :],
                bc[:],
            )
            if dbg is not None and nb_of == 0 and p == 0 and e == 1:
                nc.sync.dma_start(dbg["out00"][:, :], outHT_t[p][:].bitcast(F32))

        # The attention stream is software-pipelined one group deep: the
        # Tensor queue per group is [S(g), inject/norm, PV(g-1)], so the exp
        # of group g runs on ScalarE while the Tensor engine streams S(g+1)
        # and PV(g-1) — PV(g) never waits on its own exp in queue order.
        pend_norm = None
        inject = []
        outHT_prev = None
        outHT_by_nb = {}
        pv_by_pair = {}
        prev_grp = None  # (nb, p, kc2, et)

        def emit_pv(grp, tail=False):
            nbp, pp, kc2p, etp = grp
            key = (nbp, pp)
            if key not in pv_by_pair:
                pv_by_pair[key] = [
                    ps_v.tile([128, NQB], F32, tag="pv", name=f"pv{e}") for e in (0, 1)]
            pv = pv_by_pair[key]
            rec_dts = []
            for e in (0, 1):
                vstart = (2 * pp + e) * (HD + 1)
                for half in (0, 1):
                    kc = kc2p * 2 + half
                    csl = slice(half * NQB, (half + 1) * NQB)
                    nc.tensor.matmul(
                        pv[e][:], v_sb[kc][:, vstart:vstart + 128], etp[e][:, csl],
                        start=(kc == 0), stop=(kc == KCH - 1),
                    )
                if tail:
                    # no next pair: head e's den->recip->cast chain is emitted
                    # right after its PV stop so the bc matmuls unblock as
                    # early as possible; the pv data-row copies (needed only
                    # by the muls, which also wait on bc) come last
                    den = mid.tile([1, NQB], F32, tag="mid", name="den")
                    nc.vector.tensor_copy(den[:], pv[e][HD:HD + 1, :])
                    r = mid.tile([1, NQB], F32, tag="mid", name="rec")
                    nc.vector.reciprocal_approx_fast(r[:], den[:])
                    if DT is F32:
                        rec_dts.append(r)
                    else:
                        rd = mid.tile([1, NQB], DT, tag="mid", name="rec_dt")
                        nc.vector.tensor_copy(rd[:], r[:])
                        rec_dts.append(rd)
            if tail:
                pv_sbs = []
                for e in (0, 1):
                    s = mid.tile([HD, NQB], F32, tag="mid", name="pv_sb")
                    nc.vector.tensor_copy(s[:], pv[e][0:HD, :])
                    pv_sbs.append(s)
                del pv_by_pair[key]
                return (pv_sbs, rec_dts)
            if kc2p == KCH // 2 - 1:
                # pair done: free the PSUM banks fast and start the recip
                # chain on DVE. The denominator row is copied to a partition-0
                # tile (reciprocal_approx_fast NaNs on partition-offset input).
                pv_sbs, dens, recs = [], [], []
                for e in (0, 1):
                    s = mid.tile([HD, NQB], F32, tag="mid", name="pv_sb")
                    nc.vector.tensor_copy(s[:], pv[e][0:HD, :])
                    pv_sbs.append(s)
                    den = mid.tile([1, NQB], F32, tag="mid", name="den")
                    nc.vector.tensor_copy(den[:], pv[e][HD:HD + 1, :])
                    dens.append(den)
                for e in (0, 1):
                    r = mid.tile([1, NQB], F32, tag="mid", name="rec")
                    nc.vector.reciprocal_approx_fast(r[:], dens[e][:])
                    recs.append(r)
                if dbg is not None and nbp == 0 and pp == 0:
                    nc.sync.dma_start(dbg["pv00"][0:HD, :], pv_sbs[0][:])
                    nc.sync.dma_start(dbg["pv00"][HD:HD + 1, :], dens[0][:])
                    nc.sync.dma_start(dbg["pv01"][0:HD, :], pv_sbs[1][:])
                    nc.sync.dma_start(dbg["pv01"][HD:HD + 1, :], dens[1][:])
                    nc.sync.dma_start(dbg["rec00"][:, :], recs[0][:])
                del pv_by_pair[key]
                return (nbp, pp, outHT_by_nb[nbp], pv_sbs, recs)
            return None

        def stage_q(nb_s, p_s):
            # stage pair (nb_s, p_s)'s q into the zero-padded set p_s % 2
            nsl_s = slice(nb_s * NQB, (nb_s + 1) * NQB)
            for e in (0, 1):
                nc.vector.tensor_copy(
                    qz_stage[(p_s % 2) * 2 + e][64 * e:64 * e + 64, :],
                    qp[p_s][64 * e:64 * e + 64, nsl_s])

        stage_q(0, 0)
        for nb in range(NBLK):
            nsl = slice(nb * NQB, (nb + 1) * NQB)
            outHT_by_nb[nb] = [
                outp.tile([128, NQB], DT, tag="o", name=f"outHT{p}") for p in range(PAIRS)]
            if nb > 0:
                inject = make_proj_items(outHT_prev, slice((nb - 1) * NQB, nb * NQB))
            inj_steps = 0
            for p in range(PAIRS):
                for kc2 in range(KCH // 2):
                    st = [ps_s.tile([128, 2 * NQB], F32, tag="st", name="st") for _ in (0, 1)]
                    et = [big.tile([128, 2 * NQB], DT, tag="big", name="et") for _ in (0, 1)]
                    for half in (0, 1):
                        kc = kc2 * 2 + half
                        ksl = slice(kc * 128, (kc + 1) * 128)
                        csl = slice(half * NQB, (half + 1) * NQB)
                        for e in (0, 1):
                            nc.tensor.matmul(
                                st[e][:, csl],
                                kT[p][:, ksl],
                                qz_stage[(p % 2) * 2 + e][:],
                                start=True, stop=True,
                            )
                    if kc2 == 0:
                        # stage the NEXT pair's q (into the other set) while
                        # this pair's S stream runs
                        if p < PAIRS - 1:
                            stage_q(nb, p + 1)
                        elif nb < NBLK - 1:
                            stage_q(nb + 1, 0)
                    for e in (0, 1):
                        nc.scalar.activation(et[e][:], st[e][:], AF.Exp, scale=0.125)
                    if dbg is not None and nb == 0 and p == 0 and kc2 == 0:
                        nc.sync.dma_start(dbg["et000"][:, :], et[0][:].bitcast(F32))
                    # proj injection is thinned to ~1 matmul per group so the
                    # groups stay under the exp pace (2 steps make them
                    # Tensor-bound): 1 step at kc2 in {3..6}, 2 at kc2==7,
                    # and at kc2==0 only COMPLETING an open chunk. A chunk is
                    # thus never mid-flight across the kc2==2 norm slot,
                    # where finish_norm's two bc tiles join the same "acc"
                    # rotation, and its p=3 step always follows outHT[3]'s
                    # write (emission order = dependency order).
                    if kc2 == 1:
                        if pend_norm is not None:
                            finish_norm(pend_norm, 0)
                    elif kc2 == 2:
                        if pend_norm is not None:
                            finish_norm(pend_norm, 1)
                            pend_norm = None
                    elif kc2 in (3, 4, 5, 6, 7, 0):
                        if kc2 == 0:
                            steps = -(inj_steps % 4) % 4  # close open chunk
                        else:
                            steps = 2 if kc2 == 7 else 1
                        while steps and inject:
                            try:
                                next(inject[0])
                                inj_steps += 1
                                steps -= 1
                            except StopIteration:
                                inject.pop(0)
                    if prev_grp is not None:
                        done = emit_pv(prev_grp)
                        if done is not None:
                            pend_norm = done
                    prev_grp = (nb, p, kc2, et)
            # drain any proj leftovers before the next block's groups
            while inject:
                try:
                    next(inject[0])
                except StopIteration:
                    inject.pop(0)
            outHT_prev = outHT_by_nb[nb]

        # --- tail: last group's PV, last pair's normalize, last projection -
        # Three chunks' p0-p2 steps (the third borrowing a free ps_s slot)
        # keep the Tensor engine busy — no pstate-resetting gap — while the
        # last exp and the den->recip chain complete; the two bc broadcasts
        # share the other free ps_s slot, and the muls read pv from PSUM.
        tail_pools = ([(ps_acc, "acc")] * 2 + [(ps_s, "st")] + [(ps_acc, "acc")] * 5)
        tail_items = make_proj_items(
            outHT_prev, slice((NBLK - 1) * NQB, NBLK * NQB), pools=tail_pools)
        for it in tail_items[:3]:
            for _ in range(3):
                next(it)
        pv_last, rec_dts = emit_pv(prev_grp, tail=True)
        bc2 = ps_s.tile([128, 2 * NQB], F32, tag="st", name="bc_tail")
        for e in (0, 1):
            csl = slice(e * NQB, (e + 1) * NQB)
            nc.tensor.matmul(bc2[0:HD, csl], ones_m[:], rec_dts[e][:], start=True, stop=True)
            nc.vector.tensor_mul(
                outHT_prev[PAIRS - 1][64 * e:64 * e + HD, :],
                pv_last[e][0:HD, :], bc2[0:HD, csl],
            )
        for it in tail_items:
            for _ in it:
                pass


def _get_nc():
    key = MM_DT_NAME
    if key not in _BUILD_CACHE:
        _BUILD_CACHE[key] = _build(key)
    return _BUILD_CACHE[key]


def _np_dt():
    if MM_DT_NAME == "bfloat16":
        import ml_dtypes
        return ml_dtypes.bfloat16
    return np.float32


def _make_in_maps(np_inputs):
    dt = _np_dt()
    x = np.asarray(np_inputs["x"], dtype=np.float32)
    W_qkv = np.asarray(np_inputs["W_qkv"], dtype=np.float32)
    W_proj = np.asarray(np_inputs["W_proj"], dtype=np.float32)
    in_maps = []
    for c in range(NCORES):
        b, g = divmod(c, 2)
        rq = slice(g * CIN, (g + 1) * CIN)
        rk = slice(C + g * CIN, C + (g + 1) * CIN)
        rv = slice(2 * C + g * CIN, 2 * C + (g + 1) * CIN)
        in_maps.append({
            "xT": np.ascontiguousarray(x[b].T).astype(dt),
            "wqkT": np.ascontiguousarray(
                np.concatenate([W_qkv[rq], W_qkv[rk]], axis=0).T).astype(dt),
            "wvT": np.ascontiguousarray(W_qkv[rv].T).astype(dt),
            "wpT": np.ascontiguousarray(W_proj[:, g * CIN:(g + 1) * CIN].T).astype(dt),
        })
    return in_maps


def kernel(x, W_qkv, W_proj, b_proj):
    from concourse import bass_utils

    b_proj = np.asarray(b_proj, dtype=np.float32)
    nc = _get_nc()
    in_maps = _make_in_maps({"x": x, "W_qkv": W_qkv, "W_proj": W_proj})
    res = bass_utils.run_bass_kernel_spmd(nc, in_maps, core_ids=list(range(NCORES)))
    y = np.empty((B, N, C), dtype=np.float32)
    for b in range(B):
        yt = res.results[2 * b]["yT"] + res.results[2 * b + 1]["yT"]
        y[b] = yt.T
    return y + b_proj[None, None, :]



# revision 15
# speedup vs baseline: 1.0074x; 1.0074x over previous
"""Multi-head attention forward (B=4, N=2048, C=1024, H=16) on 8 Trainium2 cores.

Sharding: (batch, head-half) across 8 cores. Core c handles batch b = c//2 and
heads g*8..g*8+8 where g = c%2. Each core computes qkv for its head slice,
attention for its 8 heads, and a partial output projection over its 512
input-channel slice. The host sums the two partial projections per batch
(the tensor-parallel all-reduce) and adds b_proj.

On-chip dataflow (per core):
  - Phase A: k and v for ALL query blocks, q for block 0 only. q/k are
    produced transposed ([dims, tokens]); q is stored PACKED per head-pair
    in a 2-block ring ([128, 2*NQB]) and staged per pair into pre-zeroed
    [128, NQB] tiles so the score matmuls run at full K=128.
  - v is produced in natural [key, d] layout with a fused ones column per
    head, so the P@V matmul also produces softmax denominators (PSUM row 64).
  - Phase B is software-pipelined one group deep: the Tensor queue per
    128x2-key group g is [S(g), inject/norm slot, PV(g-1)], so the exp of
    group g (ScalarE, 1/sqrt(hd) scale folded in, no max-subtraction)
    overlaps S(g+1)/PV(g-1).
  - q for block nb+1 is DEFERRED: its 8-matmul chains (re-DMA'd x and W_q
    chunks feed them) are injected into block nb's groups together with the
    proj chains of block nb-1. This keeps the Tensor engine the pacer of
    phase B (the exp stream no longer stalls it) and removes ~22us of
    exp-idle phase A time.
  - Normalization is DEFERRED off the Tensor critical path: PSUM copy +
    reciprocal_approx_fast (DVE) at the pair boundary, then K=1 ones-matmul
    broadcasts + DVE multiplies two groups into the NEXT pair's stream.
  - Injection slots per group: kc2 in {3..6}: 3 steps, kc2==7: 2 steps,
    kc2==0: run the open chain to completion (so no PSUM acc chain is ever
    mid-flight across the kc2 in {1,2} norm slots, whose bc tiles join the
    same "acc" rotation). proj chains precede q chains in the stream, so
    proj's p=3 step always lands after outHT[3]'s norm write and q's
    re-DMA'd inputs have ~2 pairs of DMA lead time.
"""

import sys

if "/opt/trn_rl_repo" not in sys.path:
    sys.path.insert(0, "/opt/trn_rl_repo")

import numpy as np

B, N, C = 4, 2048, 1024
H, HD = 16, 64
NCORES = 8
HLOC = H // 2          # heads per core
PAIRS = HLOC // 2      # head-pair tiles per core
CIN = HLOC * HD        # 512: proj input slice per core
NQB = 512              # query-block width
NBLK = N // NQB        # 4
CCH = C // 128         # 8 contraction chunks for the projections
KCH = N // 128         # 16 key chunks

MM_DT_NAME = "float32r"  # "float32r" (tf32-class) or "bfloat16"

_BUILD_CACHE = {}


def _build(mm_dt_name):
    import concourse.mybir as mybir
    import concourse.tile as tile
    from concourse import bacc

    DT = getattr(mybir.dt, mm_dt_name)
    F32 = mybir.dt.float32
    AF = mybir.ActivationFunctionType

    nc = bacc.Bacc(None, target_bir_lowering=False)
    xT = nc.dram_tensor("xT", [C, N], DT, kind="ExternalInput")
    wqkT = nc.dram_tensor("wqkT", [C, 2 * CIN], DT, kind="ExternalInput")
    wvT = nc.dram_tensor("wvT", [C, CIN], DT, kind="ExternalInput")
    wpT = nc.dram_tensor("wpT", [CIN, C], DT, kind="ExternalInput")
    yT = nc.dram_tensor("yT", [C, N], F32, kind="ExternalOutput")

    with nc.allow_low_precision(reason="softmax intermediates kept in matmul dtype"):
        with tile.TileContext(nc) as tc:
            _emit(nc, tc, tile, mybir, DT, F32, AF, xT, wqkT, wvT, wpT, yT)
    nc.compile()
    return nc


def _emit(nc, tc, tile, mybir, DT, F32, AF, xT, wqkT, wvT, wpT, yT):
    from contextlib import ExitStack

    ctx = ExitStack()
    with ctx:
        persist = ctx.enter_context(tc.tile_pool(name="persist", bufs=1))
        # "big" slots ([128,1024]): wqk weights in phase A, then rotate to
        # et (exp) tiles in phase B.
        big = ctx.enter_context(tc.tile_pool(name="big", bufs=8))
        # "mid" slots ([*,512] = 2KB/partition): phase B normalize-chain
        # temporaries and yt staging.
        mid = ctx.enter_context(tc.tile_pool(name="mid", bufs=9))
        # "x" slots ([128,512]): phase A even-block xt tiles (odd blocks
        # borrow the then-idle mid pool), then the re-DMA'd x chunks for the
        # deferred q chains (tenant chain xt2 -> xq1 -> xq2 -> xq3).
        xpool = ctx.enter_context(tc.tile_pool(name="xpool", bufs=8))
        # "o" slots ([128,512]): wv weight chunks in phase A, outHT in phase B
        outp = ctx.enter_context(tc.tile_pool(name="outs", bufs=8))
        ps_s = ctx.enter_context(tc.tile_pool(name="ps_s", bufs=2, space="PSUM"))
        ps_v = ctx.enter_context(tc.tile_pool(name="ps_v", bufs=2, space="PSUM"))
        ps_acc = ctx.enter_context(tc.tile_pool(name="ps_acc", bufs=2, space="PSUM"))

        # --- persistent tiles ---------------------------------------------
        # q packed per pair in a 2-block ring: head 2p on partitions 0-63,
        # head 2p+1 on 64-127; column slot nb%2 holds block nb's queries.
        qp = [persist.tile([128, 2 * NQB], DT, tag=f"qp{p}", name=f"qp{p}") for p in range(PAIRS)]
        kT = [persist.tile([128, N], DT, tag=f"kT{p}", name=f"kT{p}") for p in range(PAIRS)]
        # v with a fused ones column per head: [key_chunk][128, HLOC, HD+1]
        # free size padded to even so the uint32-bitcast tail memset divides
        v_sb = [persist.tile([128, (HLOC + 1) * (HD + 1) + 1], DT, tag=f"v{kc}", name=f"v{kc}") for kc in range(KCH)]
        # K=1 broadcast stationary for the denominator-broadcast matmuls
        ones_m = persist.tile([1, HD], DT, tag="ones_m")
        ones_f32 = persist.tile([128, HLOC], F32, tag="ones_f32")
        wp_sb = [persist.tile([128, C], DT, tag=f"wp{p}", name=f"wp{p}") for p in range(PAIRS)]
        # zero-padded q staging for the score matmuls: K=64 sub-array matmuls
        # stream ~45% slower than K=128 on hw, so q is staged per pair into
        # these pre-zeroed tiles (head e on partitions 64e..64e+64, rest 0)
        # and S runs as a full-K matmul against the 2-head kT pair stationary.
        # Two ping-pong sets (even/odd pair), staged one pair ahead.
        qz_stage = [persist.tile([128, NQB], DT, tag=f"qs{i}", name=f"qs{i}") for i in range(4)]

        def qsl(nb):
            return slice((nb % 2) * NQB, (nb % 2) * NQB + NQB)

        # --- phase A DMA, split across three idle engine queues so the
        # ~650ns-per-DMA issue cost parallelizes: x on sync, wv on scalar,
        # wqk on gpsimd. wp is fetched after the phase A stream is queued.
        wv_sb = [outp.tile([128, CIN], DT, tag="o", name=f"wv{ci}") for ci in range(CCH)]
        xt_blk = {0: [xpool.tile([128, NQB], DT, tag="x", name="xt0") for _ in range(CCH)]}
        wqk_sb = [big.tile([128, 2 * CIN], DT, tag="big", name=f"wqk{ci}") for ci in range(CCH)]
        for ci in range(CCH):
            nc.sync.dma_start(wv_sb[ci][:], wvT[ci * 128:(ci + 1) * 128, :])
            nc.sync.dma_start(xt_blk[0][ci][:], xT[ci * 128:(ci + 1) * 128, 0:NQB])
        for ci in range(CCH):
            nc.sync.dma_start(wqk_sb[ci][:], wqkT[ci * 128:(ci + 1) * 128, :])

        # init constants after the DMA issues so they don't delay the wv
        # stream on the vector queue (all are needed only well into phase B)
        nc.vector.memset(ones_f32[:], 1.0)
        nc.vector.tensor_copy(ones_m[:], ones_f32[0:1, 0:1].broadcast_to((1, HD)))
        for i in range(4):
            nc.vector.memset(qz_stage[i][:].bitcast(mybir.dt.uint32), 0)
        for kc in range(KCH):
            v3 = v_sb[kc][:, 0:HLOC * (HD + 1)].rearrange("p (h d) -> p h d", h=HLOC)
            nc.vector.tensor_copy(v3[:, :, HD], ones_f32[:, 0:HLOC])
            # zero tail pad so head 7's 128-wide stationary window reads zeros
            nc.vector.memset(v_sb[kc][:, HLOC * (HD + 1):].bitcast(mybir.dt.uint32), 0)

        # --- phase A: all of k and v; q for block 0 only ------------------
        for nb in range(NBLK):
            nsl = slice(nb * NQB, (nb + 1) * NQB)
            xt = xt_blk.pop(nb)
            want = [nb + 1] if nb != 1 else [2, 3]
            for nxt_b in want:
                if nxt_b >= NBLK or nxt_b in xt_blk:
                    continue
                pool, ptag = (mid, "mid") if nxt_b % 2 else (xpool, "x")
                nxt = []
                for ci in range(CCH):
                    t = pool.tile([128, NQB], DT, tag=ptag, name="xt")
                    nc.sync.dma_start(t[:], xT[ci * 128:(ci + 1) * 128,
                                               nxt_b * NQB:(nxt_b + 1) * NQB])
                    nxt.append(t)
                xt_blk[nxt_b] = nxt
            # v first (block 0's weights arrive first)
            for j in range(NQB // 128):
                kc = nb * (NQB // 128) + j
                acc = ps_acc.tile([128, CIN], F32, tag="acc")
                for ci in range(CCH):
                    nc.tensor.matmul(
                        acc[:], xt[ci][:, j * 128:(j + 1) * 128], wv_sb[ci][:],
                        start=(ci == 0), stop=(ci == CCH - 1),
                    )
                v3 = v_sb[kc][:, 0:HLOC * (HD + 1)].rearrange("p (h d) -> p h d", h=HLOC)
                nc.vector.tensor_copy(
                    v3[:, :, 0:HD],
                    acc[:].rearrange("p (h d) -> p h d", h=HLOC),
                )
            # k (dt 4-7 -> kT); q (dt 0-3 -> qp) for block 0 only
            dts = list(range(4, 8)) + (list(range(4)) if nb == 0 else [])
            for dt_i in dts:
                acc = ps_acc.tile([128, NQB], F32, tag="acc")
                for ci in range(CCH):
                    nc.tensor.matmul(
                        acc[:], wqk_sb[ci][:, dt_i * 128:(dt_i + 1) * 128], xt[ci][:],
                        start=(ci == 0), stop=(ci == CCH - 1),
                    )
                if dt_i < PAIRS:
                    nc.vector.tensor_copy(qp[dt_i][:, qsl(nb)], acc[:])
                else:
                    nc.vector.tensor_copy(kT[dt_i - PAIRS][:, nsl], acc[:])

        # wp fetch: lands during block 0's attention, needed first at block 1
        # (proj of block 0).
        for pch in range(CIN // 128):
            nc.sync.dma_start(wp_sb[pch][:], wpT[pch * 128:(pch + 1) * 128, :])
        # W_q never changes: fetch it once into persistent tiles for the
        # deferred q chains (lands ~80us, well before the first chain)
        wq_sb = [persist.tile([128, CIN], DT, tag=f"wq{ci}", name=f"wq{ci}")
                 for ci in range(CCH)]
        for ci in range(CCH):
            nc.sync.dma_start(wq_sb[ci][:], wqkT[ci * 128:(ci + 1) * 128, 0:CIN])

        # --- phase B: attention + deferred normalize + interleaved proj/q -
        def make_proj_items(outHT_prev, nsl_prev, pools=None):
            items = []
            for ct in range(C // 128):
                def gen(ct=ct):
                    pool, tag = (pools[ct] if pools else (ps_acc, "acc"))
                    acc = pool.tile([128, NQB], F32, tag=tag, name="pacc")
                    for p in range(PAIRS):
                        nc.tensor.matmul(
                            acc[:], wp_sb[p][:, ct * 128:(ct + 1) * 128],
                            outHT_prev[p][:],
                            start=(p == 0), stop=(p == PAIRS - 1),
                        )
                        if p < PAIRS - 1:
                            yield
                    yt = xpool.tile([128, NQB], F32, tag="x", name="yt")
                    nc.vector.tensor_copy(yt[:], acc[:])
                    nc.sync.dma_start(yT[ct * 128:(ct + 1) * 128, nsl_prev], yt[:])
                    yield
                items.append(gen(ct))
            return items

        def make_q_items(nb_next, xq, wq):
            # deferred q for block nb_next: one 8-matmul chain per dt chunk,
            # fed by the re-DMA'd x / W_q tiles, copied into the qp ring.
            items = []
            for dt_i in range(PAIRS):
                def gen(dt_i=dt_i):
                    acc = ps_acc.tile([128, NQB], F32, tag="acc", name="qacc")
                    for ci in range(CCH):
                        nc.tensor.matmul(
                            acc[:], wq[ci][:, dt_i * 128:(dt_i + 1) * 128], xq[ci][:],
                            start=(ci == 0), stop=(ci == CCH - 1),
                        )
                        if ci < CCH - 1:
                            yield
                    nc.vector.tensor_copy(qp[dt_i][:, qsl(nb_next)], acc[:])
                    yield
                items.append(gen(dt_i))
            return items

        def finish_norm(pn, e):
            # one head per call so the two bc matmuls land in separate groups
            # (both stay under the exp pace). matmul dst partition base must
            # be 0, so each head gets its own [64, NQB] PSUM tile.
            nb_of, p, outHT_t, pv_sbs, recs = pn
            if DT is F32:
                rec_dt = recs[e]
            else:
                rec_dt = mid.tile([1, NQB], DT, tag="mid", name="rec_dt")
                nc.vector.tensor_copy(rec_dt[:], recs[e][:])
            bc = ps_acc.tile([HD, NQB], F32, tag="acc", name="bc")
            nc.tensor.matmul(bc[:], ones_m[:], rec_dt[:], start=True, stop=True)
            nc.vector.tensor_mul(
                outHT_t[p][64 * e:64 * e + HD, :], pv_sbs[e][0:HD, :],
                bc[:],
            )

        # The attention stream is software-pipelined one group deep: the
        # Tensor queue per group is [S(g), inject/norm, PV(g-1)], so the exp
        # of group g runs on ScalarE while the Tensor engine streams S(g+1)
        # and PV(g-1).
        pend_norm = None
        inject = []
        inj_open = 0           # steps taken in the current head item
        outHT_prev = None
        outHT_by_nb = {}
        pv_by_pair = {}
        prev_grp = None  # (nb, p, kc2, et)

        def drive_inject(steps):
            nonlocal inj_open
            while steps and inject:
                try:
                    next(inject[0])
                    inj_open += 1
                    steps -= 1
                except StopIteration:
                    inject.pop(0)
                    inj_open = 0

        def close_inject():
            # finish the open chain (if any) so no PSUM acc is mid-flight
            # across the upcoming norm slots
            nonlocal inj_open
            if inj_open == 0:
                return
            while inject:
                try:
                    next(inject[0])
                except StopIteration:
                    inject.pop(0)
                    inj_open = 0
                    return

        def emit_pv(grp, tail=False):
            nbp, pp, kc2p, etp = grp
            key = (nbp, pp)
            if key not in pv_by_pair:
                pv_by_pair[key] = [
                    ps_v.tile([128, NQB], F32, tag="pv", name=f"pv{e}") for e in (0, 1)]
            pv = pv_by_pair[key]
            rec_dts = []
            for e in (0, 1):
                vstart = (2 * pp + e) * (HD + 1)
                for half in (0, 1):
                    kc = kc2p * 2 + half
                    csl = slice(half * NQB, (half + 1) * NQB)
                    nc.tensor.matmul(
                        pv[e][:], v_sb[kc][:, vstart:vstart + 128], etp[e][:, csl],
                        start=(kc == 0), stop=(kc == KCH - 1),
                    )
                if tail:
                    # no next pair: head e's den->recip->cast chain is emitted
                    # right after its PV stop so the bc matmuls unblock as
                    # early as possible; the pv data-row copies (needed only
                    # by the muls, which also wait on bc) come last
                    den = mid.tile([1, NQB], F32, tag="mid", name="den")
                    nc.vector.tensor_copy(den[:], pv[e][HD:HD + 1, :])
                    r = mid.tile([1, NQB], F32, tag="mid", name="rec")
                    nc.vector.reciprocal_approx_fast(r[:], den[:])
                    if DT is F32:
                        rec_dts.append(r)
                    else:
                        rd = mid.tile([1, NQB], DT, tag="mid", name="rec_dt")
                        nc.vector.tensor_copy(rd[:], r[:])
                        rec_dts.append(rd)
            if tail:
                pv_sbs = []
                for e in (0, 1):
                    s = mid.tile([HD, NQB], F32, tag="mid", name="pv_sb")
                    nc.vector.tensor_copy(s[:], pv[e][0:HD, :])
                    pv_sbs.append(s)
                del pv_by_pair[key]
                return (pv_sbs, rec_dts)
            if kc2p == KCH // 2 - 1:
                # pair done: free the PSUM banks fast and start the recip
                # chain on DVE. The denominator row is copied to a partition-0
                # tile (reciprocal_approx_fast NaNs on partition-offset input).
                pv_sbs, dens, recs = [], [], []
                for e in (0, 1):
                    s = mid.tile([HD, NQB], F32, tag="mid", name="pv_sb")
                    nc.vector.tensor_copy(s[:], pv[e][0:HD, :])
                    pv_sbs.append(s)
                    den = mid.tile([1, NQB], F32, tag="mid", name="den")
                    nc.vector.tensor_copy(den[:], pv[e][HD:HD + 1, :])
                    dens.append(den)
                for e in (0, 1):
                    r = mid.tile([1, NQB], F32, tag="mid", name="rec")
                    nc.vector.reciprocal_approx_fast(r[:], dens[e][:])
                    recs.append(r)
                del pv_by_pair[key]
                return (nbp, pp, outHT_by_nb[nbp], pv_sbs, recs)
            return None

        def stage_q(nb_s, p_s):
            # stage pair (nb_s, p_s)'s q into the zero-padded set p_s % 2
            for e in (0, 1):
                nc.vector.tensor_copy(
                    qz_stage[(p_s % 2) * 2 + e][64 * e:64 * e + 64, :],
                    qp[p_s][64 * e:64 * e + 64, qsl(nb_s)])

        stage_q(0, 0)
        for nb in range(NBLK):
            nsl = slice(nb * NQB, (nb + 1) * NQB)
            outHT_by_nb[nb] = [
                outp.tile([128, NQB], DT, tag="o", name=f"outHT{p}") for p in range(PAIRS)]
            inject = []
            inj_open = 0
            if nb + 1 < NBLK:
                # re-DMA x chunks for the deferred q of block nb+1. q chains
                # go FIRST in the inject stream (pairs 0-1): their readers
                # then finish mid-block, releasing the xq ring slots early
                # for the next block's re-DMA.
                xq = []
                for ci in range(CCH):
                    t = xpool.tile([128, NQB], DT, tag="x", name="xq")
                    nc.sync.dma_start(t[:], xT[ci * 128:(ci + 1) * 128,
                                               (nb + 1) * NQB:(nb + 2) * NQB])
                    xq.append(t)
                inject += make_q_items(nb + 1, xq, wq_sb)
            if nb > 0:
                inject += make_proj_items(outHT_prev, slice((nb - 1) * NQB, nb * NQB))
            for p in range(PAIRS):
                for kc2 in range(KCH // 2):
                    st = [ps_s.tile([128, 2 * NQB], F32, tag="st", name="st") for _ in (0, 1)]
                    et = [big.tile([128, 2 * NQB], DT, tag="big", name="et") for _ in (0, 1)]
                    # e-major emission: st[0] is fully written after two
                    # matmuls, so exp(e0) starts ~230ns earlier and the next
                    # group's S never stalls on the st-slot WAR vs exp.
                    for e in (0, 1):
                        for half in (0, 1):
                            kc = kc2 * 2 + half
                            ksl = slice(kc * 128, (kc + 1) * 128)
                            csl = slice(half * NQB, (half + 1) * NQB)
                            nc.tensor.matmul(
                                st[e][:, csl],
                                kT[p][:, ksl],
                                qz_stage[(p % 2) * 2 + e][:],
                                start=True, stop=True,
                            )
                    if kc2 == 0:
                        # stage the NEXT pair's q (into the other set) while
                        # this pair's S stream runs
                        if p < PAIRS - 1:
                            stage_q(nb, p + 1)
                        elif nb < NBLK - 1:
                            stage_q(nb + 1, 0)
                    for e in (0, 1):
                        nc.scalar.activation(et[e][:], st[e][:], AF.Exp, scale=0.125)
                    # norm slots at kc2 in {1,2}; injection at kc2 in {3..7}
                    # (3 steps, 2 at kc2==7) and chain close-out at kc2==0.
                    # Block 0 pair 0 takes no injection (its q chains' re-DMA
                    # needs lead time and there is no proj yet).
                    if kc2 == 1:
                        if pend_norm is not None:
                            finish_norm(pend_norm, 0)
                    elif kc2 == 2:
                        if pend_norm is not None:
                            finish_norm(pend_norm, 1)
                            pend_norm = None
                    elif kc2 == 0:
                        close_inject()
                    elif nb == 0 and p == 0:
                        # q(1)'s re-DMA'd inputs start landing ~1.3us/chunk
                        # into block 0; drip a step per late group to shave
                        # the otherwise exp-paced pair-0 stretch
                        if kc2 >= 5:
                            drive_inject(1)
                    else:
                        drive_inject(2 if kc2 == 7 else 3)
                    if prev_grp is not None:
                        done = emit_pv(prev_grp)
                        if done is not None:
                            pend_norm = done
                    prev_grp = (nb, p, kc2, et)
            # drain any leftovers before the next block's groups
            while inject:
                try:
                    next(inject[0])
                except StopIteration:
                    inject.pop(0)
            inj_open = 0
            outHT_prev = outHT_by_nb[nb]

        # --- tail: last group's PV, last pair's normalize, last projection -
        # Three chunks' p0-p2 steps (the third borrowing a free ps_s slot)
        # keep the Tensor engine busy — no pstate-resetting gap — while the
        # last exp and the den->recip chain complete; the two bc broadcasts
        # share the other free ps_s slot, and the muls read pv from PSUM.
        tail_pools = ([(ps_acc, "acc")] * 2 + [(ps_s, "st")] + [(ps_v, "pv")] * 2
                      + [(ps_acc, "acc")] * 3)
        tail_items = make_proj_items(
            outHT_prev, slice((NBLK - 1) * NQB, NBLK * NQB), pools=tail_pools)
        for it in tail_items[:3]:
            for _ in range(3):
                next(it)
        pv_last, rec_dts = emit_pv(prev_grp, tail=True)
        bc2 = ps_s.tile([128, 2 * NQB], F32, tag="st", name="bc_tail")
        for e in (0, 1):
            csl = slice(e * NQB, (e + 1) * NQB)
            nc.tensor.matmul(bc2[0:HD, csl], ones_m[:], rec_dts[e][:], start=True, stop=True)
            nc.vector.tensor_mul(
                outHT_prev[PAIRS - 1][64 * e:64 * e + HD, :],
                pv_last[e][0:HD, :], bc2[0:HD, csl],
            )
        for it in tail_items:
            for _ in it:
                pass


def _get_nc():
    key = MM_DT_NAME
    if key not in _BUILD_CACHE:
        _BUILD_CACHE[key] = _build(key)
    return _BUILD_CACHE[key]


def _np_dt():
    if MM_DT_NAME == "bfloat16":
        import ml_dtypes
        return ml_dtypes.bfloat16
    return np.float32


def _make_in_maps(np_inputs):
    dt = _np_dt()
    x = np.asarray(np_inputs["x"], dtype=np.float32)
    W_qkv = np.asarray(np_inputs["W_qkv"], dtype=np.float32)
    W_proj = np.asarray(np_inputs["W_proj"], dtype=np.float32)
    in_maps = []
    for c in range(NCORES):
        b, g = divmod(c, 2)
        rq = slice(g * CIN, (g + 1) * CIN)
        rk = slice(C + g * CIN, C + (g + 1) * CIN)
        rv = slice(2 * C + g * CIN, 2 * C + (g + 1) * CIN)
        in_maps.append({
            "xT": np.ascontiguousarray(x[b].T).astype(dt),
            "wqkT": np.ascontiguousarray(
                np.concatenate([W_qkv[rq], W_qkv[rk]], axis=0).T).astype(dt),
            "wvT": np.ascontiguousarray(W_qkv[rv].T).astype(dt),
            "wpT": np.ascontiguousarray(W_proj[:, g * CIN:(g + 1) * CIN].T).astype(dt),
        })
    return in_maps


def kernel(x, W_qkv, W_proj, b_proj):
    from concourse import bass_utils

    b_proj = np.asarray(b_proj, dtype=np.float32)
    nc = _get_nc()
    in_maps = _make_in_maps({"x": x, "W_qkv": W_qkv, "W_proj": W_proj})
    res = bass_utils.run_bass_kernel_spmd(nc, in_maps, core_ids=list(range(NCORES)))
    y = np.empty((B, N, C), dtype=np.float32)
    for b in range(B):
        yt = res.results[2 * b]["yT"] + res.results[2 * b + 1]["yT"]
        y[b] = yt.T
    return y + b_proj[None, None, :]
